# revision 1
# baseline (speedup 1.0000x reference)
"""Chamfer distance kernel for Trainium2 (8 NeuronCores, SPMD).

Problem: points_src/points_trg [16, 4096, 3] f32.
  D[b,i,j] = ||x_i||^2 + ||y_j||^2 - 2 x_i.y_j
  returns (min_i D, min_j D)  — two [16, 4096] f32 arrays.

Strategy (v3 — negated pipeline, 3-engine split, no transposes):
  - Data-parallel over batch: 2 batches per core.
  - The device computes NEGATED distances: the host negates the A
    operand of the K=13 augmented fp32r matmul, so PSUM holds -D and
    every min becomes a max.  Outputs are negated back on the host.
  - Per i-tile [128 i, 4096 j]: 8 fp32r matmuls into two PSUM half
    tiles [128, 2048] f32 (4 banks each, bufs=2 -> all 8 banks).
  - Readout/convert f32->f16: ACT copies half 0 and all but the last
    FX columns of half 1; DVE reads the FX tail via a fused
    tensor_scalar (PSUM f32 -> SBUF f16 copy + row-max accum in one
    1x op).  FX balances ACT (~3.68us/tile) against DVE (~3.67).
  - Row-max: 4x-mode DVE tensor_scalar (bypass, accum op max) over
    the ACT-read ranges, accumulated into [128,1] f32 slots of FC;
    no reduction tree.  One tiny X-axis reduce per batch merges the
    3 partial slots.
  - Col-max: DVE TT-max accumulator G [128, 4096] f16 (2x mode),
    split h0/h1 so each half folds as soon as its readout lands.
  - Col partition-reduce: GPSIMD cross-lane tensor_reduce (axis=C,
    op=max) straight to a [1, 4096] f32 row, DMA'd to the output —
    no PE transposes, no PSUM round-trip.  Chunked after the last
    i-tile's fold so it pipelines with the fold chain.
  - PE pre-ramp: dummy matmuls on zeroed tiles during the input DMA
    raise the p-state so real matmuls run at peak from tile 0.

Engine busy (cost model): ACT ~235us, DVE ~234us, PE ~110us,
Pool ~12us; wall 249us/core vs 319us baseline.
"""

import sys

import numpy as np

for _p in ("/opt/trn_rl_repo",):
    if _p not in sys.path:
        sys.path.insert(0, _p)

import concourse.bass as bass
import concourse.tile as tile
from concourse import mybir
from concourse.bass_utils import run_bass_kernel_spmd

F32 = mybir.dt.float32
F32R = mybir.dt.float32r
F16 = mybir.dt.float16
MAX = mybir.AluOpType.max
BYP = mybir.AluOpType.bypass

B, N, C = 16, 4096, 3
NCORES = 8
BPC = B // NCORES          # batches per core
K = 13                     # augmented contraction length
NIT = N // 128             # i-tiles per batch (32)
HW = N // 2                # PSUM half width (2048)
QW = N // 4                # col-reduce chunk width (1024)

FX = 136                   # trailing columns DVE fuse-reads each i-tile

_MAX_WAITS = 1             # this walrus build allows 1 sync wait / instruction
_DMA = "sync"              # DMA issue engine: HWDGE via sync queue
NEG_INF = -3.0e38


def _split_excess_waits(nc):
    """Move excess sync waits onto same-engine NOPs placed just before."""
    for bb in nc.main_func.blocks:
        il = bb.instructions
        i = 0
        while i < len(il):
            inst = il[i]
            si = inst.sync_info
            if si is not None and si.on_wait and len(si.on_wait) > _MAX_WAITS:
                waits = list(si.on_wait)
                extra, keep = waits[:-_MAX_WAITS], waits[-_MAX_WAITS:]
                nops = []
                for k in range(0, len(extra), _MAX_WAITS):
                    chunk = extra[k:k + _MAX_WAITS]
                    nop = mybir.InstNoOp(
                        name=f"{inst.name}-wsplit{k}",
                        engine=inst.engine,
                        bass_nofuse=True,
                        sync_info=mybir.SyncInfo(on_wait=chunk, on_update=[]),
                    )
                    nc.register_instruction(nop, overwrite=True)
                    nops.append(nop)
                inst.sync_info = mybir.SyncInfo(
                    on_wait=keep, on_update=list(si.on_update))
                for j, nop in enumerate(nops):
                    il.insert(i + j, nop)
                i += len(nops)
            i += 1


def _round11(x):
    """Round to the fp32r grid: 11 explicit mantissa bits, RN."""
    x = np.asarray(x, np.float64)
    m, e = np.frexp(x)
    step = np.ldexp(1.0, e - 12)
    with np.errstate(invalid="ignore"):
        r = np.round(x / np.where(step == 0, 1.0, step)) * step
    return np.where(x == 0.0, 0.0, r)


def _build_aug(x, y):
    """Host-side augmented operands.  x,y: [B, N, 3] f32.

    Returns A, Bm: [B, K, N] f32 on the fp32r grid with
    sum_k A[k,i]*Bm[k,j] = -(||x_i||^2 + ||y_j||^2 - 2 x_i.y_j):
    the A side is negated so the device computes -D and reduces with
    max instead of min.
    """
    x = np.asarray(x, np.float64)
    y = np.asarray(y, np.float64)
    A = np.zeros((B, K, N), np.float64)
    Bm = np.zeros((B, K, N), np.float64)

    x1 = _round11(x)
    x2 = _round11(x - x1)
    t = -2.0 * y
    t1 = _round11(t)
    t2 = _round11(t - t1)
    for c in range(C):
        A[:, 3 * c + 0] = x1[:, :, c]
        A[:, 3 * c + 1] = x1[:, :, c]
        A[:, 3 * c + 2] = x2[:, :, c]
        Bm[:, 3 * c + 0] = t1[:, :, c]
        Bm[:, 3 * c + 1] = t2[:, :, c]
        Bm[:, 3 * c + 2] = t1[:, :, c]

    s = np.sum(x * x, axis=-1)
    s1 = _round11(s)
    s2 = _round11(s - s1)
    q = np.sum(y * y, axis=-1)
    q1 = _round11(q)
    q2 = _round11(q - q1)
    A[:, 9] = s1
    A[:, 10] = s2
    A[:, 11] = 1.0
    A[:, 12] = 1.0
    Bm[:, 9] = 1.0
    Bm[:, 10] = 1.0
    Bm[:, 11] = q1
    Bm[:, 12] = q2
    return (-A).astype(np.float32), Bm.astype(np.float32)


def _trace():
    """Build the SPMD per-core program.  Each core: BPC batches."""
    nc = bass.Bass()
    a_in = nc.declare_dram_parameter("a", [BPC, K, N], F32R, isOutput=False)
    b_in = nc.declare_dram_parameter("bm", [BPC, K, N], F32R, isOutput=False)
    omin1 = nc.declare_dram_parameter("omin1", [BPC, N], F32, isOutput=True)
    omin2 = nc.declare_dram_parameter("omin2", [BPC, N], F32, isOutput=True)

    with tile.TileContext(nc) as tc:
        with (
            tc.tile_pool(name="inp", bufs=1) as inp,
            tc.tile_pool(name="work", bufs=2) as work,
            tc.tile_pool(name="spool", bufs=4) as spool,
            tc.tile_pool(name="scr", bufs=2) as scr,
            tc.tile_pool(name="mm", bufs=2, space="PSUM") as mmp,
        ):
            NCH = 4
            CW = N // NCH
            ta, tb = [], []
            for b in range(BPC):
                t1 = inp.tile([K, N], F32R, tag=f"ta{b}")
                t2 = inp.tile([K, N], F32R, tag=f"tb{b}")
                ta.append(t1)
                tb.append(t2)
            # Pre-ramp the PE: a few matmuls on zeroed tiles raise the
            # p-state while the input DMAs are in flight, so the first
            # real matmuls run at mid rather than low speed.
            dum = inp.tile([13, 512], F16, tag="dum")
            nc.vector.memset(dum[:], 0.0)
            rpm = mmp.tile([128, HW], F32, tag="pm")
            for r in range(4):
                nc.tensor.matmul(rpm[:, 512 * (r % 2):512 * (r % 2 + 1)],
                                 dum[:, 0:128], dum[:],
                                 start=True, stop=True)
            # i-tile 0 needs only the first 128 cols of ta[0] and the
            # first tb chunk: issue them on two different DMA queues so
            # they land in parallel and the PE starts ~immediately.
            nc.gpsimd.dma_start(out=ta[0][:, 0:128], in_=a_in[0][:, 0:128])
            getattr(nc, _DMA).dma_start(out=tb[0][:, 0:CW], in_=b_in[0][:, 0:CW])
            for ch in range(1, NCH):
                sl = slice(CW * ch, CW * (ch + 1))
                getattr(nc, _DMA).dma_start(out=tb[0][:, sl], in_=b_in[0][:, sl])
            getattr(nc, _DMA).dma_start(out=ta[0][:, 128:N], in_=a_in[0][:, 128:N])
            for ch in range(NCH):
                sl = slice(CW * ch, CW * (ch + 1))
                getattr(nc, _DMA).dma_start(out=tb[1][:, sl], in_=b_in[1][:, sl])
            getattr(nc, _DMA).dma_start(out=ta[1][:], in_=a_in[1])

            for b in range(BPC):
                G = work.tile([128, N], F16, tag="G")
                # row-max partials per i-tile: [:, it, 0] = half 0,
                # [:, it, 1] = ACT part of half 1, [:, it, 2] = the
                # DVE-fused trailing FX columns.  Slot 3 unused pad.
                FC = work.tile([128, NIT, 4], F32, tag="FC")
                nc.vector.memset(FC[:], NEG_INF)
                rows = work.tile([128, NIT], F32, tag="rows")
                par = work.tile([1, N], F32, tag="par")

                for it in range(NIT):
                    lhsT = ta[b][:, 128 * it:128 * (it + 1)]
                    last = it == NIT - 1
                    S = spool.tile([128, N], F16, tag="S")
                    junk = scr.tile([128, N], F16, tag="junk")
                    for h in range(2):
                        pm = mmp.tile([128, HW], F32, tag="pm")
                        for m in range(4):
                            j0 = HW * h + 512 * m
                            nc.tensor.matmul(
                                pm[:, 512 * m:512 * (m + 1)],
                                lhsT,
                                tb[b][:, j0:j0 + 512],
                                start=True, stop=True)
                        if h == 0:
                            if it == 0:
                                # first tile of the batch: copy in two
                                # chunks so the DVE pipeline starts a
                                # quarter earlier (row-max partial for
                                # q0 parks in the spare FC slot 3).
                                nc.scalar.copy(S[:, 0:QW], pm[:, 0:QW])
                                nc.vector.tensor_scalar(
                                    junk[:, 0:QW], S[:, 0:QW], 0.0, None,
                                    BYP, MAX, accum_out=FC[:, it, 3:4])
                                nc.vector.tensor_copy(G[:, 0:QW],
                                                      S[:, 0:QW])
                                nc.scalar.copy(S[:, QW:HW], pm[:, QW:HW])
                                nc.vector.tensor_scalar(
                                    junk[:, QW:HW], S[:, QW:HW], 0.0,
                                    None, BYP, MAX,
                                    accum_out=FC[:, it, 0:1])
                                nc.vector.tensor_copy(G[:, QW:HW],
                                                      S[:, QW:HW])
                                continue
                            nc.scalar.copy(S[:, 0:HW], pm[:])
                            if not last:
                                nc.vector.tensor_tensor(
                                    G[:, 0:HW], G[:, 0:HW], S[:, 0:HW],
                                    MAX)
                            else:
                                # final fold chunked; each chunk goes
                                # straight into the GPSIMD cross-
                                # partition max reduce.
                                for jp in range(2):
                                    sl = slice(QW * jp, QW * (jp + 1))
                                    nc.vector.tensor_tensor(
                                        G[:, sl], G[:, sl], S[:, sl], MAX)
                                    nc.gpsimd.tensor_reduce(
                                        par[:, sl], G[:, sl],
                                        axis=mybir.AxisListType.C, op=MAX)
                            # row-max of half 0 (4x) overlaps ACT's
                            # copy of half 1 (deferred on the last tile
                            # in favour of the fold chain).
                            if not last:
                                nc.vector.tensor_scalar(
                                    junk[:, 0:HW], S[:, 0:HW], 0.0, None,
                                    BYP, MAX, accum_out=FC[:, it, 0:1])
                        else:
                            nc.scalar.copy(
                                S[:, HW:N - FX], pm[:, 0:HW - FX])
                            # fused readout of the FX tail: PSUM f32 ->
                            # SBUF f16 copy + row-max accum in one op.
                            nc.vector.tensor_scalar(
                                S[:, N - FX:N], pm[:, HW - FX:HW], 0.0,
                                None, BYP, MAX, accum_out=FC[:, it, 2:3])
                            if it == 0:
                                nc.vector.tensor_copy(
                                    G[:, HW:], S[:, HW:])
                            elif not last:
                                nc.vector.tensor_tensor(
                                    G[:, HW:], G[:, HW:], S[:, HW:], MAX)
                            else:
                                # feed the cross-partition reduce ASAP;
                                # the tile's row-max runs after so the
                                # GPSIMD tail starts as early as it can.
                                for jp in range(2, 4):
                                    sl = slice(QW * jp, QW * (jp + 1))
                                    nc.vector.tensor_tensor(
                                        G[:, sl], G[:, sl], S[:, sl], MAX)
                                    nc.gpsimd.tensor_reduce(
                                        par[:, sl], G[:, sl],
                                        axis=mybir.AxisListType.C, op=MAX)
                                getattr(nc, _DMA).dma_start(
                                    out=omin1[b].rearrange(
                                        "(o k) -> o k", o=1),
                                    in_=par[0:1, :])
                            nc.vector.tensor_scalar(
                                junk[:, HW:N - FX], S[:, HW:N - FX], 0.0,
                                None, BYP, MAX, accum_out=FC[:, it, 1:2])
                    if last:
                        # row-max of half 0 was deferred on the last
                        # tile; run it now, after the fold chain.
                        nc.vector.tensor_scalar(
                            junk[:, 0:HW], S[:, 0:HW], 0.0, None,
                            BYP, MAX, accum_out=FC[:, it, 0:1])

                # row-max partials -> negated row-min result
                nc.vector.tensor_reduce(
                    rows[:], FC[:], axis=mybir.AxisListType.X, op=MAX)
                # outputs: [128, 32] where [p, q] = out[128*q + p]
                # rows[p, it] -> omin2[128*it + p]: single strided DMA,
                # no transpose needed.
                getattr(nc, _DMA).dma_start(
                    out=omin2[b].rearrange("(c p) -> p c", p=128),
                    in_=rows[:])

    _split_excess_waits(nc)
    return nc


_NC_CACHE = None


def _get_nc():
    global _NC_CACHE
    if _NC_CACHE is None:
        _NC_CACHE = _trace()
    return _NC_CACHE


def _run(points_src, points_trg, trace=False, trace_kwargs=None):
    x = np.asarray(points_src, np.float32)
    y = np.asarray(points_trg, np.float32)
    assert x.shape == (B, N, C) and y.shape == (B, N, C)
    A, Bm = _build_aug(x, y)
    in_maps = [
        {"a": np.ascontiguousarray(A[BPC * i:BPC * (i + 1)]),
         "bm": np.ascontiguousarray(Bm[BPC * i:BPC * (i + 1)])}
        for i in range(NCORES)
    ]
    res = run_bass_kernel_spmd(
        _get_nc(), in_maps, list(range(NCORES)), trace=trace,
        **(trace_kwargs or {}))
    # device computed maxes of -D: negate back to mins of D
    min1 = -np.concatenate(
        [res.results[i]["omin1"] for i in range(NCORES)], axis=0)
    min2 = -np.concatenate(
        [res.results[i]["omin2"] for i in range(NCORES)], axis=0)
    return (min1, min2), res


def kernel(points_src, points_trg):
    (min1, min2), _ = _run(points_src, points_trg)
    return min1, min2



# revision 5
# speedup vs baseline: 1.1414x; 1.1414x over previous
"""Chamfer distance kernel for Trainium2 (8 NeuronCores, SPMD).

Problem: points_src/points_trg [16, 4096, 3] f32.
  D[b,i,j] = ||x_i||^2 + ||y_j||^2 - 2 x_i.y_j
  returns (min_i D, min_j D)  — two [16, 4096] f32 arrays.

Strategy (v6 — single-reader PSUM tiles, 3-engine readout + Pool
column-stripe reduces):
  - Data-parallel over batch: 2 batches per core.  Device computes
    NEGATED distances (host negates the A operand of the K=13
    augmented fp32r matmul) so every min becomes a max; host negates
    the outputs back.
  - Per i-tile [128 i, 4096 j]: 8 fp32r matmuls into THREE PSUM
    tiles pm0 [128,1536], pm1 [128,1536], pm2 [128,1024]
    (3+3+2 = 8 banks, single buffered).  Each PSUM tile has exactly
    ONE reader, so the tile-granular release chains never serialize
    readers on different engines (that cross-engine chain cost the
    v4/v5 designs ~500ns/tile).
  - Readout split to balance ACT/DVE/Pool busy:
      ACT  converts [0:1536) and [1536:3072) f32->f16 into S.
      DVE  fused tensor_scalar converts [3072:4096) + row-max accum;
           4x row-max over ACT's cols [0:3072); 2x TT-max fold of
           [FOLD0:N) into G.
      Pool (gpsimd) per-tile cross-partition max (tensor_reduce
           axis=C) over stripe [0:FOLD0) -> row `it` of
           PP[NIT, FOLD0]; the batch-end reduce over PP gives those
           columns' col-max, replacing their fold entirely.
  - Batch end: Pool reduces PP -> [1, FOLD0] (2 chunks) and
    G[:, FOLD0:N] -> [1, N-FOLD0] (2 chunks, kicked right after the
    last tile's fold chunks), forming par [1, N] -> one DMA.
    Row-max partials FC [128, NIT, 2] X-reduce -> rows [128, NIT]
    -> strided DMA (rows[p, q] = out[128 q + p]).
  - PE pre-ramp: dummy matmuls on zeroed tiles during the input DMA
    raise the p-state so real matmuls run at peak from tile 0.

Cost model per tile: DVE ~3.15us, Pool ~3.12us, ACT ~3.04us,
PE ~1.71us.
"""

import sys

import numpy as np

for _p in ("/opt/trn_rl_repo",):
    if _p not in sys.path:
        sys.path.insert(0, _p)

import concourse.bass as bass
import concourse.tile as tile
from concourse import mybir
from concourse.bass_utils import run_bass_kernel_spmd

F32 = mybir.dt.float32
F32R = mybir.dt.float32r
F16 = mybir.dt.float16
MAX = mybir.AluOpType.max
BYP = mybir.AluOpType.bypass
AXC = mybir.AxisListType.C

B, N, C = 16, 4096, 3
NCORES = 8
BPC = B // NCORES          # batches per core
K = 13                     # augmented contraction length
NIT = N // 128             # i-tiles per batch (32)

W0 = 1536                  # pm0 width (3 banks) — ACT-1
W1 = 1536                  # pm1 width (3 banks) — ACT-2
W2 = 1024                  # pm2 width (2 banks) — DVE fused
E1 = W0 + W1               # 3072 = end of ACT-converted cols
FOLD0 = 2112               # cols [0:FOLD0) col-reduced per-tile by Pool
FMID = E1                  # last-tile fold chunk boundary

_MAX_WAITS = 1             # this walrus build allows 1 sync wait / instruction
_DMA = "sync"              # DMA issue engine: HWDGE via sync queue
NEG_INF = -3.0e38


def _split_excess_waits(nc):
    """Move excess sync waits onto same-engine NOPs placed just before."""
    for bb in nc.main_func.blocks:
        il = bb.instructions
        i = 0
        while i < len(il):
            inst = il[i]
            si = inst.sync_info
            if si is not None and si.on_wait and len(si.on_wait) > _MAX_WAITS:
                waits = list(si.on_wait)
                extra, keep = waits[:-_MAX_WAITS], waits[-_MAX_WAITS:]
                nops = []
                for k in range(0, len(extra), _MAX_WAITS):
                    chunk = extra[k:k + _MAX_WAITS]
                    nop = mybir.InstNoOp(
                        name=f"{inst.name}-wsplit{k}",
                        engine=inst.engine,
                        bass_nofuse=True,
                        sync_info=mybir.SyncInfo(on_wait=chunk, on_update=[]),
                    )
                    nc.register_instruction(nop, overwrite=True)
                    nops.append(nop)
                inst.sync_info = mybir.SyncInfo(
                    on_wait=keep, on_update=list(si.on_update))
                for j, nop in enumerate(nops):
                    il.insert(i + j, nop)
                i += len(nops)
            i += 1


def _round11(x):
    """Round to the fp32r grid: 11 explicit mantissa bits, RN."""
    x = np.asarray(x, np.float64)
    m, e = np.frexp(x)
    step = np.ldexp(1.0, e - 12)
    with np.errstate(invalid="ignore"):
        r = np.round(x / np.where(step == 0, 1.0, step)) * step
    return np.where(x == 0.0, 0.0, r)


def _build_aug(x, y):
    """Host-side augmented operands.  x,y: [B, N, 3] f32.

    Returns A, Bm: [B, K, N] f32 on the fp32r grid with
    sum_k A[k,i]*Bm[k,j] = -(||x_i||^2 + ||y_j||^2 - 2 x_i.y_j):
    the A side is negated so the device computes -D and reduces with
    max instead of min.
    """
    x = np.asarray(x, np.float64)
    y = np.asarray(y, np.float64)
    A = np.zeros((B, K, N), np.float64)
    Bm = np.zeros((B, K, N), np.float64)

    x1 = _round11(x)
    x2 = _round11(x - x1)
    t = -2.0 * y
    t1 = _round11(t)
    t2 = _round11(t - t1)
    for c in range(C):
        A[:, 3 * c + 0] = x1[:, :, c]
        A[:, 3 * c + 1] = x1[:, :, c]
        A[:, 3 * c + 2] = x2[:, :, c]
        Bm[:, 3 * c + 0] = t1[:, :, c]
        Bm[:, 3 * c + 1] = t2[:, :, c]
        Bm[:, 3 * c + 2] = t1[:, :, c]

    s = np.sum(x * x, axis=-1)
    s1 = _round11(s)
    s2 = _round11(s - s1)
    q = np.sum(y * y, axis=-1)
    q1 = _round11(q)
    q2 = _round11(q - q1)
    A[:, 9] = s1
    A[:, 10] = s2
    A[:, 11] = 1.0
    A[:, 12] = 1.0
    Bm[:, 9] = 1.0
    Bm[:, 10] = 1.0
    Bm[:, 11] = q1
    Bm[:, 12] = q2
    return (-A).astype(np.float32), Bm.astype(np.float32)


def _trace():
    """Build the SPMD per-core program.  Each core: BPC batches."""
    nc = bass.Bass()
    a_in = nc.declare_dram_parameter("a", [BPC, K, N], F32R, isOutput=False)
    b_in = nc.declare_dram_parameter("bm", [BPC, K, N], F32R, isOutput=False)
    id_in = nc.declare_dram_parameter("ident", [128, 128], F16, isOutput=False)
    omin1 = nc.declare_dram_parameter("omin1", [BPC, N], F32, isOutput=True)
    omin2 = nc.declare_dram_parameter("omin2", [BPC, N], F32, isOutput=True)

    with tile.TileContext(nc) as tc:
        with (
            tc.tile_pool(name="inp", bufs=1) as inp,
            tc.tile_pool(name="work", bufs=2) as work,
            tc.tile_pool(name="spool", bufs=3) as spool,
            tc.tile_pool(name="scr", bufs=2) as scr,
            tc.tile_pool(name="mm", bufs=1, space="PSUM") as mmp,
        ):
            NCH = 4
            CW = N // NCH
            ta, tb = [], []
            for b in range(BPC):
                t1 = inp.tile([K, N], F32R, tag=f"ta{b}")
                t2 = inp.tile([K, N], F32R, tag=f"tb{b}")
                ta.append(t1)
                tb.append(t2)
            # Pre-ramp the PE: a few matmuls on zeroed tiles raise the
            # p-state while the input DMAs are in flight, so the first
            # real matmuls run at mid rather than low speed.
            dum = inp.tile([13, 512], F16, tag="dum")
            ident = inp.tile([128, 128], F16, tag="ident")
            # i-tile 0 needs only the first 128 cols of ta[0] and the
            # first tb stripe: issue them on two different DMA queues so
            # they land in parallel and the PE starts ~immediately.
            nc.gpsimd.dma_start(out=ta[0][:, 0:128], in_=a_in[0][:, 0:128])
            nc.vector.memset(dum[:], 0.0)
            rpm = mmp.tile([128, W2], F32, tag="pm2")
            for r in range(4):
                nc.tensor.matmul(rpm[:, 512 * (r % 2):512 * (r % 2 + 1)],
                                 dum[:, 0:128], dum[:],
                                 start=True, stop=True)
            getattr(nc, _DMA).dma_start(out=tb[0][:, 0:W0], in_=b_in[0][:, 0:W0])
            getattr(nc, _DMA).dma_start(out=tb[0][:, W0:E1], in_=b_in[0][:, W0:E1])
            getattr(nc, _DMA).dma_start(out=tb[0][:, E1:N], in_=b_in[0][:, E1:N])
            getattr(nc, _DMA).dma_start(out=ta[0][:, 128:N], in_=a_in[0][:, 128:N])
            for ch in range(2):
                sl = slice(N // 2 * ch, N // 2 * (ch + 1))
                getattr(nc, _DMA).dma_start(out=tb[1][:, sl], in_=b_in[1][:, sl])
            getattr(nc, _DMA).dma_start(out=ta[1][:], in_=a_in[1])
            nc.gpsimd.dma_start(out=ident[:], in_=id_in[:])

            for b in range(BPC):
                if b > 0:
                    # keep the PE p-state hot across the batch boundary:
                    # the previous batch's last ACT copy gates the next
                    # matmul by ~3.5us, enough for the p-state to drop.
                    wpm = mmp.tile([128, W2], F32, tag="pm2")
                    for r in range(12):
                        nc.tensor.matmul(
                            wpm[:, 512 * (r % 2):512 * (r % 2 + 1)],
                            dum[:, 0:128], dum[:], start=True, stop=True)
                G = work.tile([128, N], F16, tag="G")   # cols [FOLD0:N)
                PP = work.tile([NIT, FOLD0], F16, tag="PP")
                # row-max partials per i-tile: slot 0 = ACT cols
                # [0:E1), slot 1 = DVE-fused cols [E1:N).
                FC = work.tile([128, NIT, 2], F32, tag="FC")
                nc.vector.memset(FC[:], NEG_INF)
                rows = work.tile([128, NIT], F32, tag="rows")
                par = work.tile([1, N], F32, tag="par")

                for it in range(NIT):
                    lhsT = ta[b][:, 128 * it:128 * (it + 1)]
                    last = it == NIT - 1
                    S = spool.tile([128, N], F16, tag="S")
                    junk = scr.tile([128, E1], F16, tag="junk")

                    # pm0 [0:W0) -> ACT-1; pm1 [W0:E1) -> ACT-2;
                    # pm2 [E1:N) -> DVE fused.  One reader per tile.
                    pm0 = mmp.tile([128, W0], F32, tag="pm0")
                    for m in range(W0 // 512):
                        j0 = 512 * m
                        nc.tensor.matmul(
                            pm0[:, j0:j0 + 512], lhsT,
                            tb[b][:, j0:j0 + 512], start=True, stop=True)
                    nc.scalar.copy(S[:, 0:W0], pm0[:])
                    pm1 = mmp.tile([128, W1], F32, tag="pm1")
                    for m in range(W1 // 512):
                        j0 = W0 + 512 * m
                        nc.tensor.matmul(
                            pm1[:, 512 * m:512 * (m + 1)], lhsT,
                            tb[b][:, j0:j0 + 512], start=True, stop=True)
                    nc.scalar.copy(S[:, W0:E1], pm1[:])
                    pm2 = mmp.tile([128, W2], F32, tag="pm2")
                    for m in range(W2 // 512):
                        j0 = E1 + 512 * m
                        nc.tensor.matmul(
                            pm2[:, 512 * m:512 * (m + 1)], lhsT,
                            tb[b][:, j0:j0 + 512], start=True, stop=True)
                    # fused convert + row-max accum of the tail
                    nc.vector.tensor_scalar(
                        S[:, E1:N], pm2[:], 0.0, None,
                        BYP, MAX, accum_out=FC[:, it, 1:2])

                    # Pool col-stripe: cross-partition max.  The HW
                    # requires reduce outputs to start at partition 0,
                    # so reduce into a staging row and DMA it (SBUF to
                    # SBUF, idle SP queue) into row `it` of PP.
                    PPt = spool.tile([1, FOLD0], F16, tag="PPt")
                    nc.gpsimd.tensor_reduce(
                        PPt[:], S[:, 0:FOLD0], axis=AXC, op=MAX)
                    getattr(nc, _DMA).dma_start(
                        out=PP[it:it + 1, :], in_=PPt[:])

                    if not last:
                        # fold, then the 4x row-max of ACT's cols
                        if it == 0:
                            nc.vector.tensor_copy(G[:, FOLD0:N],
                                                  S[:, FOLD0:N])
                        else:
                            nc.vector.tensor_tensor(
                                G[:, FOLD0:N], G[:, FOLD0:N],
                                S[:, FOLD0:N], MAX)
                        nc.vector.tensor_scalar(
                            junk[:], S[:, 0:E1], 0.0, None,
                            BYP, MAX, accum_out=FC[:, it, 0:1])
                    elif b == 0:
                        # last tile: chunk the fold so each chunk's
                        # G-reduce starts as soon as it lands; row-max
                        # deferred behind the fold chain.
                        for (j0, j1) in ((FOLD0, FMID), (FMID, N)):
                            nc.vector.tensor_tensor(
                                G[:, j0:j1], G[:, j0:j1], S[:, j0:j1], MAX)
                            nc.gpsimd.tensor_reduce(
                                par[0:1, j0:j1], G[:, j0:j1],
                                axis=AXC, op=MAX)
                        nc.vector.tensor_scalar(
                            junk[:], S[:, 0:E1], 0.0, None,
                            BYP, MAX, accum_out=FC[:, it, 0:1])
                    else:
                        # batch-1 last tile: fold in two pieces so the
                        # PE transposes of the finished G ranges start
                        # while the rest still folds; row-max + FC merge
                        # right after so the rows DMA (the largest
                        # output DMA) overlaps the pool_max finals.
                        nc.vector.tensor_tensor(
                            G[:, FOLD0:E1], G[:, FOLD0:E1],
                            S[:, FOLD0:E1], MAX)
                        nc.vector.tensor_tensor(
                            G[:, E1:N], G[:, E1:N], S[:, E1:N], MAX)
                        nc.vector.tensor_scalar(
                            junk[:], S[:, 0:E1], 0.0, None,
                            BYP, MAX, accum_out=FC[:, it, 0:1])
                        nc.vector.tensor_reduce(
                            rows[:], FC[:], axis=mybir.AxisListType.X,
                            op=MAX)
                        getattr(nc, _DMA).dma_start(
                            out=omin2[b].rearrange("(c p) -> p c", p=128),
                            in_=rows[:])

                if b == 0:
                    # batch 0: Pool finals — absorbed by Pool's steady
                    # slack while batch 1 runs.
                    PH = FOLD0 // 2
                    for (j0, j1) in ((0, PH), (PH, FOLD0)):
                        nc.gpsimd.tensor_reduce(
                            par[0:1, j0:j1], PP[:, j0:j1], axis=AXC, op=MAX)
                    getattr(nc, _DMA).dma_start(
                        out=omin1[b].rearrange("(o k) -> o k", o=1),
                        in_=par[0:1, :])
                else:
                    # batch 1 (program tail): Pool is backlogged, so the
                    # cross-partition finals go through PE transposes +
                    # DVE pool_max, both idle here.  rows/omin2 already
                    # emitted inside the last tile.
                    NCP = FOLD0 // 128          # full PP chunks (16)
                    PPR = FOLD0 - NCP * 128     # ragged tail cols (64)
                    NCG = (N - FOLD0) // 128    # full G chunks (15)
                    GR = (N - FOLD0) - NCG * 128
                    G0 = FOLD0 + NCG * 128      # 4032
                    NG1 = (E1 - FOLD0) // 128   # chunks ready after
                    # fold-a (7 with FOLD0=2112: cols [2112:3008])
                    # PP transposes first: PP is complete once the Pool
                    # stripe of tile 31 lands, typically before the
                    # folds finish.
                    tpp32 = mmp.tile([128, W0], F32, tag="pm0")
                    tpp = tpp32[:].bitcast(F16)
                    for c in range(NCP):
                        nc.tensor.transpose(
                            tpp[:, NIT * c:NIT * (c + 1)],
                            PP[:, 128 * c:128 * (c + 1)],
                            ident[0:NIT, 0:NIT])
                    nc.tensor.transpose(
                        tpp[0:PPR, NIT * NCP:NIT * (NCP + 1)],
                        PP[:, 128 * NCP:FOLD0], ident[0:NIT, 0:NIT])
                    MP = work.tile([128, NCP + 1], F32, tag="MP")
                    nc.vector.tensor_reduce(
                        MP[:],
                        tpp[:, 0:NIT * (NCP + 1)].rearrange(
                            "p (c w) -> p c w", w=NIT),
                        axis=mybir.AxisListType.X, op=MAX)
                    getattr(nc, _DMA).dma_start(
                        out=omin1[b][0:128 * NCP].rearrange(
                            "(c p) -> p c", p=128),
                        in_=MP[:, 0:NCP])
                    nc.scalar.dma_start(
                        out=omin1[b][128 * NCP:FOLD0].rearrange(
                            "(p o) -> p o", o=1),
                        in_=MP[0:PPR, NCP:NCP + 1])
                    # G region: chunks [0:NG1) ready after fold-a,
                    # the rest after fold-b
                    tpg32 = mmp.tile([128, W1], F32, tag="pm1")
                    tpg = tpg32[:].bitcast(F16)
                    for c in range(NCG):
                        j0 = FOLD0 + 128 * c
                        nc.tensor.transpose(
                            tpg[:, 128 * c:128 * (c + 1)],
                            G[:, j0:j0 + 128], ident[:])
                    nc.tensor.transpose(
                        tpg[0:GR, 128 * NCG:128 * (NCG + 1)],
                        G[:, G0:N], ident[:])
                    MG = work.tile([128, NCG + 1], F32, tag="MG")
                    nc.vector.tensor_reduce(
                        MG[:, 0:NG1],
                        tpg[:, 0:128 * NG1].rearrange(
                            "p (c w) -> p c w", w=128),
                        axis=mybir.AxisListType.X, op=MAX)
                    nc.vector.tensor_reduce(
                        MG[:, NG1:],
                        tpg[:, 128 * NG1:128 * (NCG + 1)].rearrange(
                            "p (c w) -> p c w", w=128),
                        axis=mybir.AxisListType.X, op=MAX)
                    getattr(nc, _DMA).dma_start(
                        out=omin1[b][FOLD0:G0].rearrange(
                            "(c p) -> p c", p=128),
                        in_=MG[:, 0:NCG])
                    nc.scalar.dma_start(
                        out=omin1[b][G0:N].rearrange("(p o) -> p o", o=1),
                        in_=MG[0:GR, NCG:NCG + 1])

                if b == 0:
                    # row-max partials -> negated row-min result
                    nc.vector.tensor_reduce(
                        rows[:], FC[:], axis=mybir.AxisListType.X, op=MAX)
                    # outputs: [128, 32] where [p, q] = out[128*q + p]
                    getattr(nc, _DMA).dma_start(
                        out=omin2[b].rearrange("(c p) -> p c", p=128),
                        in_=rows[:])

    _split_excess_waits(nc)
    return nc


_NC_CACHE = None


def _get_nc():
    global _NC_CACHE
    if _NC_CACHE is None:
        _NC_CACHE = _trace()
    return _NC_CACHE


def _run(points_src, points_trg, trace=False, trace_kwargs=None):
    x = np.asarray(points_src, np.float32)
    y = np.asarray(points_trg, np.float32)
    assert x.shape == (B, N, C) and y.shape == (B, N, C)
    A, Bm = _build_aug(x, y)
    ident = np.eye(128, dtype=np.float16)
    in_maps = [
        {"a": np.ascontiguousarray(A[BPC * i:BPC * (i + 1)]),
         "bm": np.ascontiguousarray(Bm[BPC * i:BPC * (i + 1)]),
         "ident": ident}
        for i in range(NCORES)
    ]
    res = run_bass_kernel_spmd(
        _get_nc(), in_maps, list(range(NCORES)), trace=trace,
        **(trace_kwargs or {}))
    # device computed maxes of -D: negate back to mins of D
    min1 = -np.concatenate(
        [res.results[i]["omin1"] for i in range(NCORES)], axis=0)
    min2 = -np.concatenate(
        [res.results[i]["omin2"] for i in range(NCORES)], axis=0)
    return (min1, min2), res


def kernel(points_src, points_trg):
    (min1, min2), _ = _run(points_src, points_trg)
    return min1, min2


# revision 7
# speedup vs baseline: 1.1417x; 1.0003x over previous
"""Chamfer distance kernel for Trainium2 (8 NeuronCores, SPMD).

Problem: points_src/points_trg [16, 4096, 3] f32.
  D[b,i,j] = ||x_i||^2 + ||y_j||^2 - 2 x_i.y_j
  returns (min_i D, min_j D)  — two [16, 4096] f32 arrays.

Strategy (v6 — single-reader PSUM tiles, 3-engine readout + Pool
column-stripe reduces):
  - Data-parallel over batch: 2 batches per core.  Device computes
    NEGATED distances (host negates the A operand of the K=13
    augmented fp32r matmul) so every min becomes a max; host negates
    the outputs back.
  - Per i-tile [128 i, 4096 j]: 8 fp32r matmuls into THREE PSUM
    tiles pm0 [128,1536], pm1 [128,1536], pm2 [128,1024]
    (3+3+2 = 8 banks, single buffered).  Each PSUM tile has exactly
    ONE reader, so the tile-granular release chains never serialize
    readers on different engines (that cross-engine chain cost the
    v4/v5 designs ~500ns/tile).
  - Readout split to balance ACT/DVE/Pool busy:
      ACT  converts [0:1536) and [1536:3072) f32->f16 into S.
      DVE  fused tensor_scalar converts [3072:4096) + row-max accum;
           4x row-max over ACT's cols [0:3072); 2x TT-max fold of
           [FOLD0:N) into G.
      Pool (gpsimd) per-tile cross-partition max (tensor_reduce
           axis=C) over stripe [0:FOLD0) -> row `it` of
           PP[NIT, FOLD0]; the batch-end reduce over PP gives those
           columns' col-max, replacing their fold entirely.
  - Batch end: Pool reduces PP -> [1, FOLD0] (2 chunks) and
    G[:, FOLD0:N] -> [1, N-FOLD0] (2 chunks, kicked right after the
    last tile's fold chunks), forming par [1, N] -> one DMA.
    Row-max partials FC [128, NIT, 2] X-reduce -> rows [128, NIT]
    -> strided DMA (rows[p, q] = out[128 q + p]).
  - PE pre-ramp: dummy matmuls on zeroed tiles during the input DMA
    raise the p-state so real matmuls run at peak from tile 0.

Cost model per tile: DVE ~3.15us, Pool ~3.12us, ACT ~3.04us,
PE ~1.71us.
"""

import sys

import numpy as np

for _p in ("/opt/trn_rl_repo",):
    if _p not in sys.path:
        sys.path.insert(0, _p)

import concourse.bass as bass
import concourse.tile as tile
from concourse import mybir
from concourse.bass_utils import run_bass_kernel_spmd

F32 = mybir.dt.float32
F32R = mybir.dt.float32r
F16 = mybir.dt.float16
MAX = mybir.AluOpType.max
BYP = mybir.AluOpType.bypass
AXC = mybir.AxisListType.C

B, N, C = 16, 4096, 3
NCORES = 8
BPC = B // NCORES          # batches per core
K = 13                     # augmented contraction length
NIT = N // 128             # i-tiles per batch (32)

W0 = 1536                  # pm0 width (3 banks) — ACT-1
W1 = 1536                  # pm1 width (3 banks) — ACT-2
W2 = 1024                  # pm2 width (2 banks) — DVE fused
E1 = W0 + W1               # 3072 = end of ACT-converted cols
FOLD0 = 2112               # cols [0:FOLD0) col-reduced per-tile by Pool
FMID = E1                  # last-tile fold chunk boundary

_MAX_WAITS = 1             # this walrus build allows 1 sync wait / instruction
_DMA = "sync"              # DMA issue engine: HWDGE via sync queue
NEG_INF = -3.0e38


def _split_excess_waits(nc):
    """Move excess sync waits onto same-engine NOPs placed just before."""
    for bb in nc.main_func.blocks:
        il = bb.instructions
        i = 0
        while i < len(il):
            inst = il[i]
            si = inst.sync_info
            if si is not None and si.on_wait and len(si.on_wait) > _MAX_WAITS:
                waits = list(si.on_wait)
                extra, keep = waits[:-_MAX_WAITS], waits[-_MAX_WAITS:]
                nops = []
                for k in range(0, len(extra), _MAX_WAITS):
                    chunk = extra[k:k + _MAX_WAITS]
                    nop = mybir.InstNoOp(
                        name=f"{inst.name}-wsplit{k}",
                        engine=inst.engine,
                        bass_nofuse=True,
                        sync_info=mybir.SyncInfo(on_wait=chunk, on_update=[]),
                    )
                    nc.register_instruction(nop, overwrite=True)
                    nops.append(nop)
                inst.sync_info = mybir.SyncInfo(
                    on_wait=keep, on_update=list(si.on_update))
                for j, nop in enumerate(nops):
                    il.insert(i + j, nop)
                i += len(nops)
            i += 1


def _round11(x):
    """Round to the fp32r grid: 11 explicit mantissa bits, RN."""
    x = np.asarray(x, np.float64)
    m, e = np.frexp(x)
    step = np.ldexp(1.0, e - 12)
    with np.errstate(invalid="ignore"):
        r = np.round(x / np.where(step == 0, 1.0, step)) * step
    return np.where(x == 0.0, 0.0, r)


def _build_aug(x, y):
    """Host-side augmented operands.  x,y: [B, N, 3] f32.

    Returns A, Bm: [B, K, N] f32 on the fp32r grid with
    sum_k A[k,i]*Bm[k,j] = -(||x_i||^2 + ||y_j||^2 - 2 x_i.y_j):
    the A side is negated so the device computes -D and reduces with
    max instead of min.
    """
    x = np.asarray(x, np.float64)
    y = np.asarray(y, np.float64)
    A = np.zeros((B, K, N), np.float64)
    Bm = np.zeros((B, K, N), np.float64)

    x1 = _round11(x)
    x2 = _round11(x - x1)
    t = -2.0 * y
    t1 = _round11(t)
    t2 = _round11(t - t1)
    for c in range(C):
        A[:, 3 * c + 0] = x1[:, :, c]
        A[:, 3 * c + 1] = x1[:, :, c]
        A[:, 3 * c + 2] = x2[:, :, c]
        Bm[:, 3 * c + 0] = t1[:, :, c]
        Bm[:, 3 * c + 1] = t2[:, :, c]
        Bm[:, 3 * c + 2] = t1[:, :, c]

    s = np.sum(x * x, axis=-1)
    s1 = _round11(s)
    s2 = _round11(s - s1)
    q = np.sum(y * y, axis=-1)
    q1 = _round11(q)
    q2 = _round11(q - q1)
    A[:, 9] = s1
    A[:, 10] = s2
    A[:, 11] = 1.0
    A[:, 12] = 1.0
    Bm[:, 9] = 1.0
    Bm[:, 10] = 1.0
    Bm[:, 11] = q1
    Bm[:, 12] = q2
    return (-A).astype(np.float32), Bm.astype(np.float32)


def _trace():
    """Build the SPMD per-core program.  Each core: BPC batches."""
    nc = bass.Bass()
    a_in = nc.declare_dram_parameter("a", [BPC, K, N], F32R, isOutput=False)
    b_in = nc.declare_dram_parameter("bm", [BPC, K, N], F32R, isOutput=False)
    id_in = nc.declare_dram_parameter("ident", [128, 128], F16, isOutput=False)
    omin1 = nc.declare_dram_parameter("omin1", [BPC, N], F32, isOutput=True)
    omin2 = nc.declare_dram_parameter("omin2", [BPC, N], F32, isOutput=True)

    with tile.TileContext(nc) as tc:
        with (
            tc.tile_pool(name="inp", bufs=1) as inp,
            tc.tile_pool(name="work", bufs=2) as work,
            tc.tile_pool(name="spool", bufs=3) as spool,
            tc.tile_pool(name="scr", bufs=2) as scr,
            tc.tile_pool(name="mm", bufs=1, space="PSUM") as mmp,
        ):
            NCH = 4
            CW = N // NCH
            ta, tb = [], []
            for b in range(BPC):
                t1 = inp.tile([K, N], F32R, tag=f"ta{b}")
                t2 = inp.tile([K, N], F32R, tag=f"tb{b}")
                ta.append(t1)
                tb.append(t2)
            # Pre-ramp the PE: a few matmuls on zeroed tiles raise the
            # p-state while the input DMAs are in flight, so the first
            # real matmuls run at mid rather than low speed.
            dum = inp.tile([13, 512], F16, tag="dum")
            ident = inp.tile([128, 128], F16, tag="ident")
            # i-tile 0 needs only the first 128 cols of ta[0] and the
            # first tb stripe: issue them on two different DMA queues so
            # they land in parallel and the PE starts ~immediately.
            nc.gpsimd.dma_start(out=ta[0][:, 0:128], in_=a_in[0][:, 0:128])
            nc.vector.memset(dum[:], 0.0)
            rpm = mmp.tile([128, W2], F32, tag="pm2")
            for r in range(4):
                nc.tensor.matmul(rpm[:, 512 * (r % 2):512 * (r % 2 + 1)],
                                 dum[:, 0:128], dum[:],
                                 start=True, stop=True)
            getattr(nc, _DMA).dma_start(out=tb[0][:, 0:W0], in_=b_in[0][:, 0:W0])
            getattr(nc, _DMA).dma_start(out=tb[0][:, W0:E1], in_=b_in[0][:, W0:E1])
            getattr(nc, _DMA).dma_start(out=tb[0][:, E1:N], in_=b_in[0][:, E1:N])
            getattr(nc, _DMA).dma_start(out=ta[0][:, 128:N], in_=a_in[0][:, 128:N])
            for ch in range(2):
                sl = slice(N // 2 * ch, N // 2 * (ch + 1))
                getattr(nc, _DMA).dma_start(out=tb[1][:, sl], in_=b_in[1][:, sl])
            getattr(nc, _DMA).dma_start(out=ta[1][:], in_=a_in[1])
            nc.gpsimd.dma_start(out=ident[:], in_=id_in[:])

            for b in range(BPC):
                if b > 0:
                    # keep the PE p-state hot across the batch boundary:
                    # the previous batch's last ACT copy gates the next
                    # matmul by ~3.5us, enough for the p-state to drop.
                    wpm = mmp.tile([128, W2], F32, tag="pm2")
                    for r in range(12):
                        nc.tensor.matmul(
                            wpm[:, 512 * (r % 2):512 * (r % 2 + 1)],
                            dum[:, 0:128], dum[:], start=True, stop=True)
                G = work.tile([128, N], F16, tag="G")   # cols [FOLD0:N)
                PP = work.tile([NIT, FOLD0], F16, tag="PP")
                # row-max partials per i-tile: slot 0 = ACT cols
                # [0:E1), slot 1 = DVE-fused cols [E1:N).
                FC = work.tile([128, NIT, 2], F32, tag="FC")
                nc.vector.memset(FC[:], NEG_INF)
                rows = work.tile([128, NIT], F32, tag="rows")
                par = work.tile([1, N], F32, tag="par")

                for it in range(NIT):
                    lhsT = ta[b][:, 128 * it:128 * (it + 1)]
                    last = it == NIT - 1
                    S = spool.tile([128, N], F16, tag="S")
                    junk = scr.tile([128, E1], F16, tag="junk")

                    # pm0 [0:W0) -> ACT-1; pm1 [W0:E1) -> ACT-2;
                    # pm2 [E1:N) -> DVE fused.  One reader per tile.
                    pm0 = mmp.tile([128, W0], F32, tag="pm0")
                    for m in range(W0 // 512):
                        j0 = 512 * m
                        nc.tensor.matmul(
                            pm0[:, j0:j0 + 512], lhsT,
                            tb[b][:, j0:j0 + 512], start=True, stop=True)
                    nc.scalar.copy(S[:, 0:W0], pm0[:])
                    pm1 = mmp.tile([128, W1], F32, tag="pm1")
                    for m in range(W1 // 512):
                        j0 = W0 + 512 * m
                        nc.tensor.matmul(
                            pm1[:, 512 * m:512 * (m + 1)], lhsT,
                            tb[b][:, j0:j0 + 512], start=True, stop=True)
                    nc.scalar.copy(S[:, W0:E1], pm1[:])
                    pm2 = mmp.tile([128, W2], F32, tag="pm2")
                    for m in range(W2 // 512):
                        j0 = E1 + 512 * m
                        nc.tensor.matmul(
                            pm2[:, 512 * m:512 * (m + 1)], lhsT,
                            tb[b][:, j0:j0 + 512], start=True, stop=True)
                    # fused convert + row-max accum of the tail
                    nc.vector.tensor_scalar(
                        S[:, E1:N], pm2[:], 0.0, None,
                        BYP, MAX, accum_out=FC[:, it, 1:2])

                    # Pool col-stripe: cross-partition max.  The HW
                    # requires reduce outputs to start at partition 0,
                    # so reduce into a staging row and DMA it (SBUF to
                    # SBUF, idle SP queue) into row `it` of PP.
                    PPt = spool.tile([1, FOLD0], F16, tag="PPt")
                    nc.gpsimd.tensor_reduce(
                        PPt[:], S[:, 0:FOLD0], axis=AXC, op=MAX)
                    if b == 0 or not last:
                        getattr(nc, _DMA).dma_start(
                            out=PP[it:it + 1, :], in_=PPt[:])
                    else:
                        PPl = PPt   # last stripe row, merged via PE below

                    if not last:
                        # fold, then the 4x row-max of ACT's cols
                        if it == 0:
                            nc.vector.tensor_copy(G[:, FOLD0:N],
                                                  S[:, FOLD0:N])
                        else:
                            nc.vector.tensor_tensor(
                                G[:, FOLD0:N], G[:, FOLD0:N],
                                S[:, FOLD0:N], MAX)
                        nc.vector.tensor_scalar(
                            junk[:], S[:, 0:E1], 0.0, None,
                            BYP, MAX, accum_out=FC[:, it, 0:1])
                    elif b == 0:
                        # last tile: chunk the fold so each chunk's
                        # G-reduce starts as soon as it lands; row-max
                        # deferred behind the fold chain.
                        for (j0, j1) in ((FOLD0, FMID), (FMID, N)):
                            nc.vector.tensor_tensor(
                                G[:, j0:j1], G[:, j0:j1], S[:, j0:j1], MAX)
                            nc.gpsimd.tensor_reduce(
                                par[0:1, j0:j1], G[:, j0:j1],
                                axis=AXC, op=MAX)
                        nc.vector.tensor_scalar(
                            junk[:], S[:, 0:E1], 0.0, None,
                            BYP, MAX, accum_out=FC[:, it, 0:1])
                    else:
                        # batch-1 last tile: fold in two pieces so the
                        # PE transposes of the finished G ranges start
                        # while the rest still folds; row-max + FC merge
                        # right after so the rows DMA (the largest
                        # output DMA) overlaps the pool_max finals.
                        nc.vector.tensor_tensor(
                            G[:, FOLD0:E1], G[:, FOLD0:E1],
                            S[:, FOLD0:E1], MAX)
                        nc.vector.tensor_tensor(
                            G[:, E1:N], G[:, E1:N], S[:, E1:N], MAX)
                        nc.vector.tensor_scalar(
                            junk[:], S[:, 0:E1], 0.0, None,
                            BYP, MAX, accum_out=FC[:, it, 0:1])
                        nc.vector.tensor_reduce(
                            rows[:], FC[:], axis=mybir.AxisListType.X,
                            op=MAX)
                        getattr(nc, _DMA).dma_start(
                            out=omin2[b].rearrange("(c p) -> p c", p=128),
                            in_=rows[:])

                if b == 0:
                    # batch 0: Pool finals — absorbed by Pool's steady
                    # slack while batch 1 runs.
                    PH = FOLD0 // 2
                    for (j0, j1) in ((0, PH), (PH, FOLD0)):
                        nc.gpsimd.tensor_reduce(
                            par[0:1, j0:j1], PP[:, j0:j1], axis=AXC, op=MAX)
                    getattr(nc, _DMA).dma_start(
                        out=omin1[b].rearrange("(o k) -> o k", o=1),
                        in_=par[0:1, :])
                else:
                    # batch 1 (program tail): Pool is backlogged, so the
                    # cross-partition finals go through PE transposes +
                    # DVE reduces, both idle here.  rows/omin2 already
                    # emitted inside the last tile.  G transposes are
                    # emitted FIRST: the in-order PE SEQ would otherwise
                    # park them behind the PP transposes' waits.
                    NCP = FOLD0 // 128          # full PP chunks (16)
                    PPR = FOLD0 - NCP * 128     # ragged tail cols (64)
                    NCG = (N - FOLD0) // 128    # full G chunks (15)
                    GR = (N - FOLD0) - NCG * 128
                    G0 = FOLD0 + NCG * 128      # 4032
                    NG1 = (E1 - FOLD0) // 128   # chunks ready after
                    # fold-a (7 with FOLD0=2112: cols [2112:3008])
                    tpg32 = mmp.tile([128, W1], F32, tag="pm1")
                    tpg = tpg32[:].bitcast(F16)
                    for c in range(NCG):
                        j0 = FOLD0 + 128 * c
                        nc.tensor.transpose(
                            tpg[:, 128 * c:128 * (c + 1)],
                            G[:, j0:j0 + 128], ident[:])
                    nc.tensor.transpose(
                        tpg[0:GR, 128 * NCG:128 * (NCG + 1)],
                        G[:, G0:N], ident[:])
                    # PP transposes: rows 0..30 from PP into stride-32
                    # windows (pad col 31 of each window is never read);
                    # row 31 from the staging row PPl at even stride-2
                    # columns (PSUM f16 writes must be 4B aligned).
                    tpp32 = mmp.tile([128, W0], F32, tag="pm0")
                    tpp = tpp32[:].bitcast(F16)
                    NR = NIT - 1
                    LO = NIT * (NCP + 1)        # PPl strip offset (544)
                    for c in range(NCP):
                        nc.tensor.transpose(
                            tpp[:, NIT * c:NIT * c + NR],
                            PP[0:NR, 128 * c:128 * (c + 1)],
                            ident[0:NR, 0:NR])
                        nc.tensor.transpose(
                            tpp[:, LO + 2 * c:LO + 2 * c + 1],
                            PPl[:, 128 * c:128 * (c + 1)],
                            ident[0:1, 0:1])
                    nc.tensor.transpose(
                        tpp[0:PPR, NIT * NCP:NIT * NCP + NR],
                        PP[0:NR, 128 * NCP:FOLD0], ident[0:NR, 0:NR])
                    nc.tensor.transpose(
                        tpp[0:PPR, LO + 2 * NCP:LO + 2 * NCP + 1],
                        PPl[:, 128 * NCP:FOLD0], ident[0:1, 0:1])
                    MP = work.tile([128, NCP + 1], F32, tag="MP")
                    PPlT = work.tile([128, NCP + 1], F16, tag="PPlT")
                    nc.vector.tensor_reduce(
                        MP[:],
                        tpp[:, 0:NIT * (NCP + 1)].rearrange(
                            "p (c w) -> p c w", w=NIT)[:, :, 0:NR],
                        axis=mybir.AxisListType.X, op=MAX)
                    nc.vector.tensor_copy(
                        PPlT[:],
                        tpp[:, LO:LO + 2 * (NCP + 1)].rearrange(
                            "p (c w) -> p c w", w=2)[:, :, 0:1])
                    nc.vector.tensor_tensor(MP[:], MP[:], PPlT[:], MAX)
                    getattr(nc, _DMA).dma_start(
                        out=omin1[b][0:128 * NCP].rearrange(
                            "(c p) -> p c", p=128),
                        in_=MP[:, 0:NCP])
                    nc.scalar.dma_start(
                        out=omin1[b][128 * NCP:FOLD0].rearrange(
                            "(p o) -> p o", o=1),
                        in_=MP[0:PPR, NCP:NCP + 1])
                    MG = work.tile([128, NCG + 1], F32, tag="MG")
                    nc.vector.tensor_reduce(
                        MG[:, 0:NG1],
                        tpg[:, 0:128 * NG1].rearrange(
                            "p (c w) -> p c w", w=128),
                        axis=mybir.AxisListType.X, op=MAX)
                    nc.vector.tensor_reduce(
                        MG[:, NG1:],
                        tpg[:, 128 * NG1:128 * (NCG + 1)].rearrange(
                            "p (c w) -> p c w", w=128),
                        axis=mybir.AxisListType.X, op=MAX)
                    getattr(nc, _DMA).dma_start(
                        out=omin1[b][FOLD0:G0].rearrange(
                            "(c p) -> p c", p=128),
                        in_=MG[:, 0:NCG])
                    nc.scalar.dma_start(
                        out=omin1[b][G0:N].rearrange("(p o) -> p o", o=1),
                        in_=MG[0:GR, NCG:NCG + 1])

                if b == 0:
                    # row-max partials -> negated row-min result
                    nc.vector.tensor_reduce(
                        rows[:], FC[:], axis=mybir.AxisListType.X, op=MAX)
                    # outputs: [128, 32] where [p, q] = out[128*q + p]
                    getattr(nc, _DMA).dma_start(
                        out=omin2[b].rearrange("(c p) -> p c", p=128),
                        in_=rows[:])

    _split_excess_waits(nc)
    return nc


_NC_CACHE = None


def _get_nc():
    global _NC_CACHE
    if _NC_CACHE is None:
        _NC_CACHE = _trace()
    return _NC_CACHE


def _run(points_src, points_trg, trace=False, trace_kwargs=None):
    x = np.asarray(points_src, np.float32)
    y = np.asarray(points_trg, np.float32)
    assert x.shape == (B, N, C) and y.shape == (B, N, C)
    A, Bm = _build_aug(x, y)
    ident = np.eye(128, dtype=np.float16)
    in_maps = [
        {"a": np.ascontiguousarray(A[BPC * i:BPC * (i + 1)]),
         "bm": np.ascontiguousarray(Bm[BPC * i:BPC * (i + 1)]),
         "ident": ident}
        for i in range(NCORES)
    ]
    res = run_bass_kernel_spmd(
        _get_nc(), in_maps, list(range(NCORES)), trace=trace,
        **(trace_kwargs or {}))
    # device computed maxes of -D: negate back to mins of D
    min1 = -np.concatenate(
        [res.results[i]["omin1"] for i in range(NCORES)], axis=0)
    min2 = -np.concatenate(
        [res.results[i]["omin2"] for i in range(NCORES)], axis=0)
    return (min1, min2), res


def kernel(points_src, points_trg):
    (min1, min2), _ = _run(points_src, points_trg)
    return min1, min2


# revision 12
# speedup vs baseline: 1.1506x; 1.0078x over previous
"""Chamfer distance kernel for Trainium2 (8 NeuronCores, SPMD).

Problem: points_src/points_trg [16, 4096, 3] f32.
  D[b,i,j] = ||x_i||^2 + ||y_j||^2 - 2 x_i.y_j
  returns (min_i D, min_j D)  — two [16, 4096] f32 arrays.

Strategy (v6 — single-reader PSUM tiles, 3-engine readout + Pool
column-stripe reduces):
  - Data-parallel over batch: 2 batches per core.  Device computes
    NEGATED distances (host negates the A operand of the K=13
    augmented fp32r matmul) so every min becomes a max; host negates
    the outputs back.
  - Per i-tile [128 i, 4096 j]: 8 fp32r matmuls into THREE PSUM
    tiles pm0 [128,1536], pm1 [128,1536], pm2 [128,1024]
    (3+3+2 = 8 banks, single buffered).  Each PSUM tile has exactly
    ONE reader, so the tile-granular release chains never serialize
    readers on different engines (that cross-engine chain cost the
    v4/v5 designs ~500ns/tile).
  - Readout split to balance ACT/DVE/Pool busy:
      ACT  converts [0:1536) and [1536:3072) f32->f16 into S.
      DVE  fused tensor_scalar converts [3072:4096) + row-max accum;
           4x row-max over ACT's cols [0:3072); 2x TT-max fold of
           [FOLD0:N) into G.
      Pool (gpsimd) per-tile cross-partition max (tensor_reduce
           axis=C) over stripe [0:FOLD0) -> row `it` of
           PP[NIT, FOLD0]; the batch-end reduce over PP gives those
           columns' col-max, replacing their fold entirely.
  - Batch end: Pool reduces PP -> [1, FOLD0] (2 chunks) and
    G[:, FOLD0:N] -> [1, N-FOLD0] (2 chunks, kicked right after the
    last tile's fold chunks), forming par [1, N] -> one DMA.
    Row-max partials FC [128, NIT, 2] X-reduce -> rows [128, NIT]
    -> strided DMA (rows[p, q] = out[128 q + p]).
  - PE pre-ramp: dummy matmuls on zeroed tiles during the input DMA
    raise the p-state so real matmuls run at peak from tile 0.

Cost model per tile: DVE ~3.15us, Pool ~3.12us, ACT ~3.04us,
PE ~1.71us.
"""

import sys

import numpy as np

for _p in ("/opt/trn_rl_repo",):
    if _p not in sys.path:
        sys.path.insert(0, _p)

import concourse.bass as bass
import concourse.tile as tile
from concourse import mybir
from concourse.bass_utils import run_bass_kernel_spmd

F32 = mybir.dt.float32
F32R = mybir.dt.float32r
F16 = mybir.dt.float16
MAX = mybir.AluOpType.max
BYP = mybir.AluOpType.bypass
AXC = mybir.AxisListType.C

B, N, C = 16, 4096, 3
NCORES = 8
BPC = B // NCORES          # batches per core
K = 13                     # augmented contraction length
NIT = N // 128             # i-tiles per batch (32)

W0 = 1536                  # pm0 width (3 banks) — ACT-1
W1 = 1536                  # pm1 width (3 banks) — ACT-2
W2 = 1024                  # pm2 width (2 banks) — DVE fused
E1 = W0 + W1               # 3072 = end of ACT-converted cols
FOLD0 = 2112               # cols [0:FOLD0) col-reduced per-tile by Pool
FMID = E1                  # last-tile fold chunk boundary

_MAX_WAITS = 1             # this walrus build allows 1 sync wait / instruction
_DMA = "sync"              # DMA issue engine: HWDGE via sync queue
NEG_INF = -3.0e38


def _split_excess_waits(nc):
    """Move excess sync waits onto same-engine NOPs placed just before."""
    for bb in nc.main_func.blocks:
        il = bb.instructions
        i = 0
        while i < len(il):
            inst = il[i]
            si = inst.sync_info
            if si is not None and si.on_wait and len(si.on_wait) > _MAX_WAITS:
                waits = list(si.on_wait)
                extra, keep = waits[:-_MAX_WAITS], waits[-_MAX_WAITS:]
                nops = []
                for k in range(0, len(extra), _MAX_WAITS):
                    chunk = extra[k:k + _MAX_WAITS]
                    nop = mybir.InstNoOp(
                        name=f"{inst.name}-wsplit{k}",
                        engine=inst.engine,
                        bass_nofuse=True,
                        sync_info=mybir.SyncInfo(on_wait=chunk, on_update=[]),
                    )
                    nc.register_instruction(nop, overwrite=True)
                    nops.append(nop)
                inst.sync_info = mybir.SyncInfo(
                    on_wait=keep, on_update=list(si.on_update))
                for j, nop in enumerate(nops):
                    il.insert(i + j, nop)
                i += len(nops)
            i += 1


def _round11(x):
    """Round to the fp32r grid: 11 explicit mantissa bits, RN."""
    x = np.asarray(x, np.float64)
    m, e = np.frexp(x)
    step = np.ldexp(1.0, e - 12)
    with np.errstate(invalid="ignore"):
        r = np.round(x / np.where(step == 0, 1.0, step)) * step
    return np.where(x == 0.0, 0.0, r)


def _build_aug(x, y):
    """Host-side augmented operands.  x,y: [B, N, 3] f32.

    Returns A, Bm: [B, K, N] f32 on the fp32r grid with
    sum_k A[k,i]*Bm[k,j] = -(||x_i||^2 + ||y_j||^2 - 2 x_i.y_j):
    the A side is negated so the device computes -D and reduces with
    max instead of min.
    """
    x = np.asarray(x, np.float64)
    y = np.asarray(y, np.float64)
    A = np.zeros((B, K, N), np.float64)
    Bm = np.zeros((B, K, N), np.float64)

    x1 = _round11(x)
    x2 = _round11(x - x1)
    t = -2.0 * y
    t1 = _round11(t)
    t2 = _round11(t - t1)
    for c in range(C):
        A[:, 3 * c + 0] = x1[:, :, c]
        A[:, 3 * c + 1] = x1[:, :, c]
        A[:, 3 * c + 2] = x2[:, :, c]
        Bm[:, 3 * c + 0] = t1[:, :, c]
        Bm[:, 3 * c + 1] = t2[:, :, c]
        Bm[:, 3 * c + 2] = t1[:, :, c]

    s = np.sum(x * x, axis=-1)
    s1 = _round11(s)
    s2 = _round11(s - s1)
    q = np.sum(y * y, axis=-1)
    q1 = _round11(q)
    q2 = _round11(q - q1)
    A[:, 9] = s1
    A[:, 10] = s2
    A[:, 11] = 1.0
    A[:, 12] = 1.0
    Bm[:, 9] = 1.0
    Bm[:, 10] = 1.0
    Bm[:, 11] = q1
    Bm[:, 12] = q2
    return (-A).astype(np.float32), Bm.astype(np.float32)


def _trace():
    """Build the SPMD per-core program.  Each core: BPC batches."""
    nc = bass.Bass()
    a_in = nc.declare_dram_parameter("a", [BPC, K, N], F32R, isOutput=False)
    b_in = nc.declare_dram_parameter("bm", [BPC, K, N], F32R, isOutput=False)
    id_in = nc.declare_dram_parameter("ident", [128, 128], F16, isOutput=False)
    omin1 = nc.declare_dram_parameter("omin1", [BPC, N], F32, isOutput=True)
    omin2 = nc.declare_dram_parameter("omin2", [BPC, N], F32, isOutput=True)

    with tile.TileContext(nc) as tc:
        with (
            tc.tile_pool(name="inp", bufs=1) as inp,
            tc.tile_pool(name="work", bufs=2) as work,
            tc.tile_pool(name="spool", bufs=3) as spool,
            tc.tile_pool(name="scr", bufs=2) as scr,
            tc.tile_pool(name="mm", bufs=1, space="PSUM") as mmp,
        ):
            NCH = 4
            CW = N // NCH
            ta, tb = [], []
            for b in range(BPC):
                t1 = inp.tile([K, N], F32R, tag=f"ta{b}")
                t2 = inp.tile([K, N], F32R, tag=f"tb{b}")
                ta.append(t1)
                tb.append(t2)
            # Pre-ramp the PE: a few matmuls on zeroed tiles raise the
            # p-state while the input DMAs are in flight, so the first
            # real matmuls run at mid rather than low speed.
            dum = inp.tile([13, 512], F16, tag="dum")
            ident = inp.tile([128, 128], F16, tag="ident")
            # i-tile 0 needs only the first 128 cols of ta[0] and the
            # first tb stripe: issue them on two different DMA queues so
            # they land in parallel and the PE starts ~immediately.
            nc.gpsimd.dma_start(out=ta[0][:, 0:128], in_=a_in[0][:, 0:128])
            nc.vector.memset(dum[:], 0.0)
            rpm = mmp.tile([128, W2], F32, tag="pm2")
            for r in range(4):
                nc.tensor.matmul(rpm[:, 512 * (r % 2):512 * (r % 2 + 1)],
                                 dum[:, 0:128], dum[:],
                                 start=True, stop=True)
            getattr(nc, _DMA).dma_start(out=tb[0][:, 0:W0], in_=b_in[0][:, 0:W0])
            getattr(nc, _DMA).dma_start(out=tb[0][:, W0:E1], in_=b_in[0][:, W0:E1])
            getattr(nc, _DMA).dma_start(out=tb[0][:, E1:N], in_=b_in[0][:, E1:N])
            getattr(nc, _DMA).dma_start(out=ta[0][:, 128:N], in_=a_in[0][:, 128:N])
            for ch in range(2):
                sl = slice(N // 2 * ch, N // 2 * (ch + 1))
                getattr(nc, _DMA).dma_start(out=tb[1][:, sl], in_=b_in[1][:, sl])
            getattr(nc, _DMA).dma_start(out=ta[1][:], in_=a_in[1])
            nc.gpsimd.dma_start(out=ident[:], in_=id_in[:])

            for b in range(BPC):
                if b > 0:
                    # keep the PE p-state hot across the batch boundary:
                    # the previous batch's last ACT copy gates the next
                    # matmul by ~3.5us, enough for the p-state to drop.
                    wpm = mmp.tile([128, W2], F32, tag="pm2")
                    for r in range(16):
                        nc.tensor.matmul(
                            wpm[:, 512 * (r % 2):512 * (r % 2 + 1)],
                            dum[:, 0:128], dum[:], start=True, stop=True)
                G = work.tile([128, N], F16, tag="G")   # cols [FOLD0:N)
                PP = work.tile([NIT, FOLD0], F16, tag="PP")
                # row-max partials per i-tile: slot 0 = ACT cols
                # [0:E1), slot 1 = DVE-fused cols [E1:N).
                FC = work.tile([128, NIT, 2], F32, tag="FC")
                nc.vector.memset(FC[:], NEG_INF)
                rows = work.tile([128, NIT], F32, tag="rows")
                par = work.tile([1, N], F32, tag="par")

                for it in range(NIT):
                    lhsT = ta[b][:, 128 * it:128 * (it + 1)]
                    last = it == NIT - 1
                    S = spool.tile([128, N], F16, tag="S")
                    junk = scr.tile([128, E1], F16, tag="junk")

                    # pm0 [0:W0) -> ACT-1; pm1 [W0:E1) -> ACT-2;
                    # pm2 [E1:N) -> DVE fused.  One reader per tile.
                    pm0 = mmp.tile([128, W0], F32, tag="pm0")
                    for m in range(W0 // 512):
                        j0 = 512 * m
                        nc.tensor.matmul(
                            pm0[:, j0:j0 + 512], lhsT,
                            tb[b][:, j0:j0 + 512], start=True, stop=True)
                    nc.scalar.copy(S[:, 0:W0], pm0[:])
                    pm1 = mmp.tile([128, W1], F32, tag="pm1")
                    for m in range(W1 // 512):
                        j0 = W0 + 512 * m
                        nc.tensor.matmul(
                            pm1[:, 512 * m:512 * (m + 1)], lhsT,
                            tb[b][:, j0:j0 + 512], start=True, stop=True)
                    nc.scalar.copy(S[:, W0:E1], pm1[:])
                    pm2 = mmp.tile([128, W2], F32, tag="pm2")
                    for m in range(W2 // 512):
                        j0 = E1 + 512 * m
                        nc.tensor.matmul(
                            pm2[:, 512 * m:512 * (m + 1)], lhsT,
                            tb[b][:, j0:j0 + 512], start=True, stop=True)
                    # fused convert + row-max accum of the tail
                    nc.vector.tensor_scalar(
                        S[:, E1:N], pm2[:], 0.0, None,
                        BYP, MAX, accum_out=FC[:, it, 1:2])

                    # Pool col-stripe: cross-partition max.  The HW
                    # requires reduce outputs to start at partition 0,
                    # so reduce into a staging row and DMA it (SBUF to
                    # SBUF, idle SP queue) into row `it` of PP.
                    PPt = spool.tile([1, FOLD0], F16, tag="PPt")
                    nc.gpsimd.tensor_reduce(
                        PPt[:], S[:, 0:FOLD0], axis=AXC, op=MAX)
                    if b == 0 or not last:
                        getattr(nc, _DMA).dma_start(
                            out=PP[it:it + 1, :], in_=PPt[:])
                    else:
                        PPl = PPt   # last stripe row, merged via PE below

                    if not last:
                        # fold, then the 4x row-max of ACT's cols
                        if it == 0:
                            nc.vector.tensor_copy(G[:, FOLD0:N],
                                                  S[:, FOLD0:N])
                        else:
                            nc.vector.tensor_tensor(
                                G[:, FOLD0:N], G[:, FOLD0:N],
                                S[:, FOLD0:N], MAX)
                        nc.vector.tensor_scalar(
                            junk[:], S[:, 0:E1], 0.0, None,
                            BYP, MAX, accum_out=FC[:, it, 0:1])
                    elif b == 0:
                        # last tile: chunk the fold so each chunk's
                        # G-reduce starts as soon as it lands; row-max
                        # deferred behind the fold chain.
                        for (j0, j1) in ((FOLD0, FMID), (FMID, N)):
                            nc.vector.tensor_tensor(
                                G[:, j0:j1], G[:, j0:j1], S[:, j0:j1], MAX)
                            nc.gpsimd.tensor_reduce(
                                par[0:1, j0:j1], G[:, j0:j1],
                                axis=AXC, op=MAX)
                        nc.vector.tensor_scalar(
                            junk[:], S[:, 0:E1], 0.0, None,
                            BYP, MAX, accum_out=FC[:, it, 0:1])
                    else:
                        # batch-1 last tile: fold in two pieces so the
                        # PE transposes of the finished G ranges start
                        # while the rest still folds; row-max + FC merge
                        # right after so the rows DMA (the largest
                        # output DMA) overlaps the pool_max finals.
                        nc.vector.tensor_tensor(
                            G[:, FOLD0:E1], G[:, FOLD0:E1],
                            S[:, FOLD0:E1], MAX)
                        nc.vector.tensor_tensor(
                            G[:, E1:N], G[:, E1:N], S[:, E1:N], MAX)
                        nc.vector.tensor_scalar(
                            junk[:], S[:, 0:E1], 0.0, None,
                            BYP, MAX, accum_out=FC[:, it, 0:1])
                        nc.vector.tensor_reduce(
                            rows[:], FC[:], axis=mybir.AxisListType.X,
                            op=MAX)
                        getattr(nc, _DMA).dma_start(
                            out=omin2[b].rearrange("(c p) -> p c", p=128),
                            in_=rows[:])

                if b == 0:
                    # batch 0: Pool finals — absorbed by Pool's steady
                    # slack while batch 1 runs.
                    PH = FOLD0 // 2
                    for (j0, j1) in ((0, PH), (PH, FOLD0)):
                        nc.gpsimd.tensor_reduce(
                            par[0:1, j0:j1], PP[:, j0:j1], axis=AXC, op=MAX)
                    getattr(nc, _DMA).dma_start(
                        out=omin1[b].rearrange("(o k) -> o k", o=1),
                        in_=par[0:1, :])
                else:
                    # batch 1 (program tail): Pool is backlogged, so the
                    # cross-partition finals go through PE transposes +
                    # DVE reduces, both idle here.  rows/omin2 already
                    # emitted inside the last tile.  G transposes are
                    # emitted FIRST: the in-order PE SEQ would otherwise
                    # park them behind the PP transposes' waits.
                    NCP = FOLD0 // 128          # full PP chunks (16)
                    PPR = FOLD0 - NCP * 128     # ragged tail cols (64)
                    NCG = (N - FOLD0) // 128    # full G chunks (15)
                    GR = (N - FOLD0) - NCG * 128
                    G0 = FOLD0 + NCG * 128      # 4032
                    NG1 = (E1 - FOLD0) // 128   # chunks ready after
                    # fold-a (7 with FOLD0=2112: cols [2112:3008])
                    tpg32 = mmp.tile([128, W1], F32, tag="pm1")
                    tpg = tpg32[:].bitcast(F16)
                    for c in range(NCG):
                        j0 = FOLD0 + 128 * c
                        nc.tensor.transpose(
                            tpg[:, 128 * c:128 * (c + 1)],
                            G[:, j0:j0 + 128], ident[:])
                    nc.tensor.transpose(
                        tpg[0:GR, 128 * NCG:128 * (NCG + 1)],
                        G[:, G0:N], ident[:])
                    # PP transposes: rows 0..30 from PP into stride-32
                    # windows (pad col 31 of each window is never read);
                    # row 31 from the staging row PPl at even stride-2
                    # columns (PSUM f16 writes must be 4B aligned).
                    tpp32 = mmp.tile([128, W0], F32, tag="pm0")
                    tpp = tpp32[:].bitcast(F16)
                    NR = NIT - 1
                    LO = NIT * (NCP + 1)        # PPl strip offset (544)
                    for c in range(NCP):
                        nc.tensor.transpose(
                            tpp[:, NIT * c:NIT * c + NR],
                            PP[0:NR, 128 * c:128 * (c + 1)],
                            ident[0:NR, 0:NR])
                        nc.tensor.transpose(
                            tpp[:, LO + 2 * c:LO + 2 * c + 1],
                            PPl[:, 128 * c:128 * (c + 1)],
                            ident[0:1, 0:1])
                    nc.tensor.transpose(
                        tpp[0:PPR, NIT * NCP:NIT * NCP + NR],
                        PP[0:NR, 128 * NCP:FOLD0], ident[0:NR, 0:NR])
                    nc.tensor.transpose(
                        tpp[0:PPR, LO + 2 * NCP:LO + 2 * NCP + 1],
                        PPl[:, 128 * NCP:FOLD0], ident[0:1, 0:1])
                    MP = work.tile([128, NCP + 1], F32, tag="MP")
                    PPlT = work.tile([128, NCP + 1], F16, tag="PPlT")
                    with tc.high_priority():
                        nc.vector.tensor_reduce(
                            MP[:],
                            tpp[:, 0:NIT * (NCP + 1)].rearrange(
                                "p (c w) -> p c w", w=NIT)[:, :, 0:NR],
                            axis=mybir.AxisListType.X, op=MAX)
                        nc.vector.tensor_copy(
                            PPlT[:],
                            tpp[:, LO:LO + 2 * (NCP + 1)].rearrange(
                                "p (c w) -> p c w", w=2)[:, :, 0:1])
                        nc.vector.tensor_tensor(MP[:], MP[:], PPlT[:], MAX)
                        getattr(nc, _DMA).dma_start(
                            out=omin1[b][0:128 * NCP].rearrange(
                                "(c p) -> p c", p=128),
                            in_=MP[:, 0:NCP])
                        nc.scalar.dma_start(
                            out=omin1[b][128 * NCP:FOLD0].rearrange(
                                "(p o) -> p o", o=1),
                            in_=MP[0:PPR, NCP:NCP + 1])
                    MG = work.tile([128, NCG + 1], F32, tag="MG")
                    nc.vector.tensor_reduce(
                        MG[:, 0:NG1],
                        tpg[:, 0:128 * NG1].rearrange(
                            "p (c w) -> p c w", w=128),
                        axis=mybir.AxisListType.X, op=MAX)
                    nc.vector.tensor_reduce(
                        MG[:, NG1:],
                        tpg[:, 128 * NG1:128 * (NCG + 1)].rearrange(
                            "p (c w) -> p c w", w=128),
                        axis=mybir.AxisListType.X, op=MAX)
                    getattr(nc, _DMA).dma_start(
                        out=omin1[b][FOLD0:G0].rearrange(
                            "(c p) -> p c", p=128),
                        in_=MG[:, 0:NCG])
                    nc.scalar.dma_start(
                        out=omin1[b][G0:N].rearrange("(p o) -> p o", o=1),
                        in_=MG[0:GR, NCG:NCG + 1])

                if b == 0:
                    # row-max partials -> negated row-min result
                    nc.vector.tensor_reduce(
                        rows[:], FC[:], axis=mybir.AxisListType.X, op=MAX)
                    # outputs: [128, 32] where [p, q] = out[128*q + p]
                    getattr(nc, _DMA).dma_start(
                        out=omin2[b].rearrange("(c p) -> p c", p=128),
                        in_=rows[:])

    _split_excess_waits(nc)
    return nc


_NC_CACHE = None


def _get_nc():
    global _NC_CACHE
    if _NC_CACHE is None:
        _NC_CACHE = _trace()
    return _NC_CACHE


def _run(points_src, points_trg, trace=False, trace_kwargs=None):
    x = np.asarray(points_src, np.float32)
    y = np.asarray(points_trg, np.float32)
    assert x.shape == (B, N, C) and y.shape == (B, N, C)
    A, Bm = _build_aug(x, y)
    ident = np.eye(128, dtype=np.float16)
    in_maps = [
        {"a": np.ascontiguousarray(A[BPC * i:BPC * (i + 1)]),
         "bm": np.ascontiguousarray(Bm[BPC * i:BPC * (i + 1)]),
         "ident": ident}
        for i in range(NCORES)
    ]
    res = run_bass_kernel_spmd(
        _get_nc(), in_maps, list(range(NCORES)), trace=trace,
        **(trace_kwargs or {}))
    # device computed maxes of -D: negate back to mins of D
    min1 = -np.concatenate(
        [res.results[i]["omin1"] for i in range(NCORES)], axis=0)
    min2 = -np.concatenate(
        [res.results[i]["omin2"] for i in range(NCORES)], axis=0)
    return (min1, min2), res


def kernel(points_src, points_trg):
    (min1, min2), _ = _run(points_src, points_trg)
    return min1, min2
